# revision 1
# baseline (speedup 1.0000x reference)
"""Trainium2 Bass kernel for nn_DualDescriptorTS.

Math:  Nk[b,i] = sum_{j,g} x[b,j] * P[i,j,g] * cos(2*pi*k[b]/p[i,j,g]),
       p[i,j,g] = i*1024 + j*16 + g + 2,  x = emb[token_indices].

Key identity (k = arange(B), so k_b = b = 32*h + l, h in [0,128),
l in [0,32)): by angle addition, the P-weighted phi slab of each
(i, j) pair is a small-rank product

  D_{i,j}[l, h] = stat^T mov,
  stat[(c,g), l] = {P*cos(l*th_g), -P*sin(l*th_g)},  mov[(c,g), h] =
  {cos(32h*th_g), sin(32h*th_g)},  th_g = 2*pi/p.

mov is P-independent and numerically low rank (the 16 periods of a
slab are nearly equal): mov ~ A @ Q with per-slab rank K (median 2 at
a 1% Frobenius tail).  Q ships as the matmul moving operand; P folds
into the tiny stationary factor A^T @ stat per call.

Work distribution: the unit is a chain (i, sg2, ccol) — the four slabs
j = 4*(sg2+4m)+ccol that one on-device reduction lane sums.  For each
of the 16 (sg2, ccol) positions the 64 i-chains are dealt across the
8 cores by descending weight (rank r -> core r%8, row-group r//8), so
the near-full-rank i=0 slabs spread evenly and the SPMD contract
depths (max over cores) stay tight.  The host adds the final 16
partial groups per output row using the chain map.

Device per core and row-group: 64 col-tiled K x 32 x 128 matmuls (4
PE column groups concurrent) fill PSUM [4*32 l-bands, 16 slots x 128
h]; the DVE multiplies PSUM halves by the token embeddings (fp16);
gpsimd + DVE fold 16 slots into 4 groups; [128, 512] fp16 rows DMA
out.
"""
import numpy as np
import ml_dtypes

import concourse.bacc as bacc
import concourse.tile as tile
from concourse import mybir
from concourse.bass_utils import run_bass_kernel_spmd

F32 = mybir.dt.float32
BF16 = mybir.dt.bfloat16
FP16 = mybir.dt.float16
TWO_PI = 2.0 * np.pi

M, O, B = 64, 16, 4096
NCORES = 8
NI = 8            # row-groups per core
NH, NL = 128, 32  # b = 32*h + l
TAU = 0.01        # relative Frobenius tail kept when truncating mov

_bf16 = ml_dtypes.bfloat16
_fp16 = np.float16
_nc_cache = {}
_last_results = None


def _factors():
    """P-independent SVD factors, the chain map, and contract depths.

    imap[c, rg, sg2, ccol] = i_global handled by core c, row-group rg,
    reduction lane (sg2, ccol).  KPROG[rg, j] = contract depth of the
    matmul at (rg, slot=j//4, ccol=j%4) — max slab rank over cores.
    """
    if "fac" in _nc_cache:
        return _nc_cache["fac"]
    h = np.arange(NH, dtype=np.float64)
    ig = np.arange(M, dtype=np.float64)[:, None, None]
    jg = np.arange(M, dtype=np.float64)[None, :, None]
    gg = np.arange(O, dtype=np.float64)[None, None, :]
    theta = TWO_PI / (1024.0 * ig + 16.0 * jg + gg + 2.0)
    a1 = theta[..., None] * (32.0 * h)
    mov = np.concatenate([np.cos(a1), np.sin(a1)], axis=2).reshape(M * M, 32, NH)
    U, S, Vt = np.linalg.svd(mov.astype(np.float64), full_matrices=False)
    fro = np.sqrt((S ** 2).sum(1))
    tail = np.sqrt(np.cumsum((S ** 2)[:, ::-1], axis=1))[:, ::-1] / fro[:, None]
    Ks = np.maximum(
        np.array([np.searchsorted(-tail[s], -TAU) for s in range(M * M)]), 1)
    A = (U * S[:, None, :]).astype(np.float32)              # [4096, 32, 32]

    Ksq = Ks.reshape(M, M)                                  # [i, j]
    imap = np.zeros((NCORES, NI, 4, 4), dtype=np.int64)
    for sg2 in range(4):
        for ccol in range(4):
            js = [4 * (sg2 + 4 * m) + ccol for m in range(4)]
            w = Ksq[:, js].sum(axis=1)                      # weight per i
            order = np.argsort(-w, kind="stable")           # heavy first
            for r, i in enumerate(order):
                imap[r % NCORES, r // NCORES, sg2, ccol] = i
    KPROG = np.zeros((NI, M), dtype=np.int64)               # [rg, j]
    for rg in range(NI):
        for j in range(M):
            sg2, ccol = (j // 4) % 4, j % 4
            KPROG[rg, j] = max(Ksq[imap[c, rg, sg2, ccol], j]
                               for c in range(NCORES))
    fac = (A, Vt.astype(np.float32), Ksq, imap, KPROG)
    _nc_cache["fac"] = fac
    return fac


def _build():
    if "nc" in _nc_cache:
        return _nc_cache["nc"]
    _, _, _, _, KPROG = _factors()
    nc = bacc.Bacc(target_bir_lowering=False, debug=False)
    wt_d = nc.declare_dram_parameter("wt", [32, 65536], BF16, isOutput=False)
    vt_d = nc.declare_dram_parameter("vt", [32, 16384], BF16, isOutput=False)
    xa_d = nc.declare_dram_parameter("xa", [128, 2048], FP16, isOutput=False)
    out_d = nc.declare_dram_parameter("out", [1024, 512], FP16, isOutput=True)

    with tile.TileContext(nc) as tc:
        with (
            tc.tile_pool(name="xap", bufs=1) as xpool,
            tc.tile_pool(name="wv", bufs=1) as wpool,
            tc.tile_pool(name="tmp", bufs=3) as tpool,
            tc.tile_pool(name="red", bufs=3) as rpool,
            tc.tile_pool(name="ps", bufs=4, space="PSUM") as psum,
        ):
            rgorder = [1, 2, 3, 4, 5, 6, 7, 0]
            xa = xpool.tile([128, 2048], FP16)
            # Five input transfers on three issue queues: DMA issue costs
            # ~700ns serially per engine, so light rgs 1..7 ship as one
            # strided block at their max contract depth.
            wl = wpool.tile([32, 7 * 8192], BF16, name="wl")
            vl = wpool.tile([32, 7 * 2048], BF16, name="vl")
            wh = wpool.tile([32, 8192], BF16, name="wh")
            vh = wpool.tile([32, 2048], BF16, name="vh")
            # xa first — the sync DMA queue holds only ~3 transfers in
            # flight, and every mult needs xa; then staged light chunks
            # (rows ~2 after chain balancing), then heavy rg0 quarters.
            nc.sync.dma_start(xa[:], xa_d[:])
            for rgs in ([1], [2, 3], [4, 5, 6, 7]):
                Rc = int(KPROG[rgs].max())
                a, b = rgs[0] - 1, rgs[-1]
                nc.sync.dma_start(wl[0:Rc, 8192 * a:8192 * b],
                                  wt_d[0:Rc, 8192 * (a + 1):8192 * (b + 1)])
                nc.scalar.dma_start(vl[0:Rc, 2048 * a:2048 * b],
                                    vt_d[0:Rc, 2048 * (a + 1):2048 * (b + 1)])
            Kq0 = KPROG[0].reshape(4, 16).max(axis=1)
            for q in range(4):
                K = int(Kq0[q])
                nc.sync.dma_start(wh[0:K, 2048 * q:2048 * (q + 1)],
                                  wt_d[0:K, 2048 * q:2048 * (q + 1)])
                nc.scalar.dma_start(vh[0:K, 512 * q:512 * (q + 1)],
                                    vt_d[0:K, 512 * q:512 * (q + 1)])
            wt_t = {0: (wh, 0)}
            vt_t = {0: (vh, 0)}
            for rg in range(1, NI):
                wt_t[rg] = (wl, 8192 * (rg - 1))
                vt_t[rg] = (vl, 2048 * (rg - 1))

            for n, rg in enumerate(rgorder):
                # 64 col-tiled matmuls per row-group (4 PE column groups
                # run concurrently; row-band tiling is rejected by this
                # HW path).  j = 4*slot + ccol.
                tx = tpool.tile([128, 2048], FP16, name=f"tx{rg}", tag="tx")
                for half in range(2):
                    ps = psum.tile([128, 1024], F32, tag="ps",
                                   name=f"ps{rg}_{half}")
                    for sh in range(8):
                        slot = 8 * half + sh
                        for ccol in range(4):
                            j = 4 * slot + ccol
                            K = int(KPROG[rg][j])
                            vtile, voff = vt_t[rg]
                            wtile, woff = wt_t[rg]
                            nc.tensor.matmul(
                                ps[32 * ccol:32 * ccol + 32,
                                   128 * sh:128 * sh + 128],
                                vtile[0:K, voff + 32 * j:voff + 32 * j + 32],
                                wtile[0:K, woff + 128 * j:
                                      woff + 128 * j + 128],
                                start=True, stop=True,
                                tile_position=(0, 32 * ccol))
                    nc.vector.tensor_tensor(
                        tx[:, 1024 * half:1024 * (half + 1)], ps[:, :],
                        xa[:, 1024 * half:1024 * (half + 1)],
                        mybir.AluOpType.mult)
                t1 = rpool.tile([128, 1024], FP16, name=f"t1_{rg}", tag="t1")
                eng1 = nc.gpsimd if n < NI - 1 else nc.vector
                eng1.tensor_tensor(t1[:], tx[:, 0:1024], tx[:, 1024:2048],
                                   mybir.AluOpType.add)
                t2 = rpool.tile([128, 512], FP16, name=f"t2_{rg}", tag="t2")
                eng2 = nc.vector if n % 2 == 0 else nc.gpsimd
                eng2.tensor_tensor(t2[:], t1[:, 0:512], t1[:, 512:1024],
                                   mybir.AluOpType.add)
                nc.scalar.dma_start(out_d[128 * rg:128 * (rg + 1), :], t2[:])
    nc.compile()
    _nc_cache["nc"] = nc
    return nc


def _pack_tables(P_):
    """Per-core bf16 tables following the chain map.  Slab at (rg, j)
    of core c is (i = imap[c, rg, (j//4)%4, j%4], j); rows K..KPROG are
    zero."""
    A, Vt, Ksq, imap, KPROG = _factors()
    l = np.arange(NL, dtype=np.float64)
    ig = np.arange(M, dtype=np.float64)[:, None, None]
    jg = np.arange(M, dtype=np.float64)[None, :, None]
    gg = np.arange(O, dtype=np.float64)[None, None, :]
    theta = TWO_PI / (1024.0 * ig + 16.0 * jg + gg + 2.0)
    a2 = theta[..., None] * l
    Pd = P_.astype(np.float64)
    stat = np.concatenate([Pd[..., None] * np.cos(a2),
                           -Pd[..., None] * np.sin(a2)],
                          axis=2).reshape(M * M, 32, NL).astype(np.float32)
    statp = np.matmul(A.transpose(0, 2, 1), stat)            # [4096,32,32]
    wts, vts = [], []
    for c in range(NCORES):
        wt = np.zeros((32, NI * 8192), dtype=_bf16)
        vt = np.zeros((32, NI * 2048), dtype=_bf16)
        for rg in range(NI):
            for j in range(M):
                i = imap[c, rg, (j // 4) % 4, j % 4]
                s = i * M + j
                K = int(Ksq[i, j])
                wt[0:K, 8192 * rg + NH * j:8192 * rg + NH * (j + 1)] = \
                    Vt[s][0:K].astype(_bf16)
                vt[0:K, 2048 * rg + NL * j:2048 * rg + NL * (j + 1)] = \
                    statp[s][0:K].astype(_bf16)
        wts.append(wt)
        vts.append(vt)
    return wts, vts


def _pack_x(x):
    # xa[32*ccol + l, 128*s + h] = x[32h+l, j], j = 4*s + ccol
    x4 = x.reshape(NH, NL, 16, 4)                 # [h, l, s, ccol]
    xa = np.ascontiguousarray(x4.transpose(3, 1, 2, 0)).reshape(128, 2048)
    return xa.astype(_fp16)


def _numpy_fallback(k, x, P_):
    out = np.zeros((B, M), dtype=np.float32)
    periods = (np.arange(M * M * O, dtype=np.float32) + 2.0).reshape(M, M, O)
    CH = 256
    for s0 in range(0, B, CH):
        kb = k[s0:s0 + CH].astype(np.float32)
        phi = np.cos(np.float32(TWO_PI) * kb[:, None, None, None]
                     / periods[None]).astype(np.float32)
        out[s0:s0 + CH] = np.einsum('bj,ijg,bijg->bi', x[s0:s0 + CH],
                                    P_.astype(np.float32), phi,
                                    optimize=True).astype(np.float32)
    return out


def kernel(k_tensor, token_indices, emb, P):
    global _last_results
    k = np.asarray(k_tensor, dtype=np.float32).reshape(B)
    tok = np.asarray(token_indices).astype(np.int64).reshape(B)
    emb_ = np.asarray(emb, dtype=np.float32)
    P_ = np.asarray(P, dtype=np.float32)
    x = emb_[tok]                                          # [B, 64]

    if not np.array_equal(k, np.arange(B, dtype=np.float32)):
        return _numpy_fallback(k, x, P_)

    _, _, _, imap, _ = _factors()
    wts, vts = _pack_tables(P_)
    xa = _pack_x(x)
    nc = _build()
    in_maps = [{"wt": wts[c], "vt": vts[c], "xa": xa} for c in range(NCORES)]
    res = run_bass_kernel_spmd(nc, in_maps, list(range(NCORES)))
    _last_results = res
    out = np.zeros((B, M), dtype=np.float32)
    for c in range(NCORES):
        od = res.results[c]["out"].astype(np.float32)       # [1024, 512]
        # row = 128*rg + 32*ccol + l; col = 128*sg2 + h
        part = od.reshape(NI, 4, NL, 4, NH)                 # [rg,ccol,l,sg2,h]
        for rg in range(NI):
            for ccol in range(4):
                for sg2 in range(4):
                    i = imap[c, rg, sg2, ccol]
                    out[:, i] += part[rg, ccol, :, sg2, :].T.reshape(B)
    return out



# revision 5
# speedup vs baseline: 1.0201x; 1.0201x over previous
"""Trainium2 Bass kernel for nn_DualDescriptorTS.

Math:  Nk[b,i] = sum_{j,g} x[b,j] * P[i,j,g] * cos(2*pi*k[b]/p[i,j,g]),
       p[i,j,g] = i*1024 + j*16 + g + 2,  x = emb[token_indices].

Factorization (k = arange(B), b = 32*h + l): angle addition gives each
(i, j) slab D_{i,j}[l, h] = stat^T mov with mov P-independent and
numerically low rank (exactly 2 at a 1% tail for every i >= 2 slab;
only i in {0, 1} are heavier).  mov ~ A @ Q via SVD; P folds into the
tiny stationary factor statp = A^T stat per call.

v2 design (from the v1 trace: no engine saturated; 12.6us startup DMA
stall, 25us of 512 tiny matmuls, 18us serial drain):
  * Merged matmuls: the 4 ccol slabs of a (rg, half, sh) slot run as
    ONE matmul with a block-diagonal stationary [Ktot<=128, 128] --
    PE time is free-dim bound (128 cycles/mm), so 128 mms ~ 7us vs
    512 mms ~ 27us.  Contract depth is free in time; it only costs
    LDWEIGHTS rows + DMA bytes.
  * Heavy chains (i=0, i=1) are spread one-per-core per mm slot in
    rg0, so no merged mm exceeds depth 38.
  * Chains of 2 slabs (j, j+32): one on-device fold (t1 = h0 + h1),
    host sums the 32 partials per output row.  Cuts DVE fold work 3x
    vs v1's two folds.
  * Elementwise split: scalar engine copies PSUM->SBUF fp16 (ACT
    copy), vector multiplies fp16 at 2x mode; fold split vector /
    gpsimd by columns.  Direct PSUM multiply on vector where scalar
    is the cap.
  * DMA: few large rectangles, first-needed rg's tables land first,
    outputs stream per-rg on the idle sync engine.
"""
import numpy as np
import ml_dtypes

import concourse.bacc as bacc
import concourse.tile as tile
from concourse import mybir
from concourse.bass_utils import run_bass_kernel_spmd

F32 = mybir.dt.float32
BF16 = mybir.dt.bfloat16
FP16 = mybir.dt.float16
TWO_PI = 2.0 * np.pi

M, O, B = 64, 16, 4096
NC = 8            # cores
NI = 8            # row-groups (rgs) per core
NH, NL = 128, 32  # b = 32*h + l
TAU = 0.01        # relative Frobenius tail kept when truncating mov

_bf16 = ml_dtypes.bfloat16
_fp16 = np.float16
_cache = {}
_last_results = None

# per-(rg, half) elementwise mode: 'C' = scalar copies PSUM->fp16 then
# vector multiplies fp16 (2x mode); 'D' = vector multiplies straight
# from PSUM (1x mode).  VC = fold columns done on vector (rest gpsimd).
CFG_MODE = [('C', 'D')] * NI
VC = 384
RGORDER = [1, 2, 3, 0, 4, 5, 6, 7]


def _heavy_core(i, sh, ccol):
    """Core holding chain (i in {0,1}) at cell (sh, ccol) of rg0."""
    if i == 0:
        return ccol + 4 * (sh % 2)
    return ccol + 4 * ((sh + 1) % 2)


def _factors():
    """P-independent prep: SVD factors, chain map, program depths,
    per-core base tables (mov filled), and stat scatter specs."""
    if "fac" in _cache:
        return _cache["fac"]
    h = np.arange(NH, dtype=np.float64)
    l = np.arange(NL, dtype=np.float64)
    ig = np.arange(M, dtype=np.float64)[:, None, None]
    jg = np.arange(M, dtype=np.float64)[None, :, None]
    gg = np.arange(O, dtype=np.float64)[None, None, :]
    theta = TWO_PI / (1024.0 * ig + 16.0 * jg + gg + 2.0)  # [i, j, g]
    a1 = theta[..., None] * (32.0 * h)
    mov = np.concatenate([np.cos(a1), np.sin(a1)], axis=2).reshape(
        M * M, 32, NH)
    U, S, Vt = np.linalg.svd(mov, full_matrices=False)
    fro = np.sqrt((S ** 2).sum(1))
    tail = np.sqrt(np.cumsum((S ** 2)[:, ::-1], axis=1))[:, ::-1] / fro[:, None]
    Ks = np.maximum(
        np.array([np.searchsorted(-tail[s], -TAU) for s in range(M * M)]), 1)
    Ksq = Ks.reshape(M, M)
    A = (U * S[:, None, :])                                 # [4096, 32, 32]

    # G[s][g, k, l]: statp_s[k, l] = sum_g P[i,j,g] * G_s[g, k, l]
    thl = theta.reshape(M * M, O)[:, :, None] * l           # [s, g, l]
    cl, sl = np.cos(thl), np.sin(thl)
    # A[s, g, k]*cos - A[s, 16+g, k]*sin  -> [s, g, k, l]
    Kmax01 = int(Ksq[:2].max())
    G01 = (A[:128, :O, :, None] * cl[:128, :, None, :]
           - A[:128, O:, :, None] * sl[:128, :, None, :]).astype(np.float32)
    G2 = (A[128:, :O, :2, None] * cl[128:, :, None, :]
          - A[128:, O:, :2, None] * sl[128:, :, None, :]).astype(np.float32)

    # chain map: imap[c, rg, sh, ccol] = i (chain covers j=4sh+ccol, j+32)
    imap = np.zeros((NC, NI, 8, 4), dtype=np.int64)
    for sh in range(8):
        for ccol in range(4):
            imap[_heavy_core(0, sh, ccol), 0, sh, ccol] = 0
            imap[_heavy_core(1, sh, ccol), 0, sh, ccol] = 1
            light = []
            for rg in range(NI):
                for c in range(NC):
                    if rg == 0 and (c == _heavy_core(0, sh, ccol)
                                    or c == _heavy_core(1, sh, ccol)):
                        continue
                    light.append((c, rg))
            assert len(light) == 62
            for n, (c, rg) in enumerate(light):
                imap[c, rg, sh, ccol] = n + 2

    # program contract depths DEP[rg, half, sh] (max over cores)
    DEP = np.zeros((NI, 2, 8), dtype=np.int64)
    for rg in range(NI):
        for half in range(2):
            for sh in range(8):
                d = 0
                for c in range(NC):
                    dc = sum(int(Ksq[imap[c, rg, sh, ccol],
                                     4 * (8 * half + sh) + ccol])
                             for ccol in range(4))
                    d = max(d, dc)
                DEP[rg, half, sh] = d
    assert DEP.max() <= 128

    # rg0 mm ordering by depth (desc) for the two DMA rects
    dep0 = DEP[0].reshape(16)
    order0 = np.argsort(-dep0, kind="stable")     # mm slots S=8*half+sh
    NA = 2                                        # mms in deep rect A
    HA_ROWS = int(dep0[order0[:NA]].max())
    HB_ROWS = int(dep0[order0[NA:]].max())
    pos0 = np.zeros(16, dtype=np.int64)           # slot S -> col index
    for n, Sd in enumerate(order0):
        pos0[Sd] = n

    # ---- base tables (mov filled, stat zero) + stat scatter specs ----
    # light tensors: lt1 rg1 [8, 4096]; lt2 rg2-3 [8, 8192]; lt3 rg4-7
    # [8, 16384].  Per-rg block: mov cols [0:2048] (16 mm x 128), stat
    # cols [2048:4096] (16 mm x 128 with 32ccol+l inside).
    # rg0: ha [HA_ROWS, NA*256]; hb [HB_ROWS, 14*256] -- per mm:
    # mov [dep, 128] at 256*n, stat at 256*n + 128.
    base = []      # per core dict of arrays
    spec = []      # per core: (name, flat_idx, slab_idx, k_idx)
    for c in range(NC):
        lt = [np.zeros((8, 4096 * w), dtype=_bf16) for w in (1, 2, 4)]
        ha = np.zeros((HA_ROWS, NA * 256), dtype=_bf16)
        hb = np.zeros((HB_ROWS, (16 - NA) * 256), dtype=_bf16)
        arrs = {"lt1": lt[0], "lt2": lt[1], "lt3": lt[2],
                "ha": ha, "hb": hb}
        sp = {k: ([], [], [], []) for k in arrs}
        for rg in range(NI):
            for half in range(2):
                for sh in range(8):
                    Sd = 8 * half + sh
                    if rg == 0:
                        n = pos0[Sd]
                        name = "ha" if n < NA else "hb"
                        a = arrs[name]
                        mc = 256 * (n if n < NA else n - NA)
                        sc = mc + 128
                    else:
                        name = "lt1" if rg == 1 else (
                            "lt2" if rg <= 3 else "lt3")
                        a = arrs[name]
                        off = {1: 0, 2: 0, 3: 4096, 4: 0, 5: 4096,
                               6: 8192, 7: 12288}[rg]
                        mc = off + 128 * Sd
                        sc = off + 2048 + 128 * Sd
                    r0 = 0
                    for ccol in range(4):
                        i = int(imap[c, rg, sh, ccol])
                        j = 4 * (8 * half + sh) + ccol
                        s = i * M + j
                        K = int(Ksq[i, j])
                        a[r0:r0 + K, mc:mc + 128] = \
                            Vt[s][0:K].astype(_bf16)
                        cols = a.shape[1]
                        fi, si, ki, li = sp[name]
                        for k in range(K):
                            base_flat = (r0 + k) * cols + sc + 32 * ccol
                            fi.extend(range(base_flat, base_flat + 32))
                            si.extend([s] * 32)
                            ki.extend([k] * 32)
                            li.extend(range(32))
                        r0 += K
        base.append(arrs)
        spec.append({k: (np.array(v[0], dtype=np.int64),
                         np.array(v[1], dtype=np.int64),
                         np.array(v[2], dtype=np.int64),
                         np.array(v[3], dtype=np.int64))
                     for k, v in sp.items()})

    fac = dict(Ksq=Ksq, imap=imap, DEP=DEP, G01=G01, G2=G2,
               Kmax01=Kmax01, HA=HA_ROWS, HB=HB_ROWS, NA=NA,
               pos0=pos0, base=base, spec=spec)
    _cache["fac"] = fac
    return fac


def _build():
    if "nc" in _cache:
        return _cache["nc"]
    fac = _factors()
    DEP, HA, HB, NA, pos0 = (fac["DEP"], fac["HA"], fac["HB"],
                             fac["NA"], fac["pos0"])
    nc = bacc.Bacc(target_bir_lowering=False, debug=False)
    lt1_d = nc.declare_dram_parameter("lt1", [8, 4096], BF16, isOutput=False)
    lt2_d = nc.declare_dram_parameter("lt2", [8, 8192], BF16, isOutput=False)
    lt3_d = nc.declare_dram_parameter("lt3", [8, 16384], BF16, isOutput=False)
    ha_d = nc.declare_dram_parameter("ha", [HA, NA * 256], BF16,
                                     isOutput=False)
    hb_d = nc.declare_dram_parameter("hb", [HB, (16 - NA) * 256], BF16,
                                     isOutput=False)
    xa_d = nc.declare_dram_parameter("xa", [128, 2048], FP16, isOutput=False)
    out_d = nc.declare_dram_parameter("out", [1024, 1024], FP16,
                                      isOutput=True)

    with tile.TileContext(nc) as tc:
        with (
            tc.tile_pool(name="tabs", bufs=1) as wpool,
            tc.tile_pool(name="cp", bufs=3) as cpool,
            tc.tile_pool(name="tx", bufs=4) as tpool,
            tc.tile_pool(name="t1", bufs=3) as rpool,
            tc.tile_pool(name="ps", bufs=4, space="PSUM") as psum,
        ):
            lt1 = wpool.tile([8, 4096], BF16, name="lt1")
            lt2 = wpool.tile([8, 8192], BF16, name="lt2")
            lt3 = wpool.tile([8, 16384], BF16, name="lt3")
            ha = wpool.tile([HA, NA * 256], BF16, name="ha")
            hb = wpool.tile([HB, (16 - NA) * 256], BF16, name="hb")
            xa = wpool.tile([128, 2048], FP16, name="xa")
            # input DMA: first-needed tables first; xa on a second
            # issue queue so it flows in parallel.
            nc.sync.dma_start(lt1[:], lt1_d[:])
            nc.scalar.dma_start(xa[:], xa_d[:])
            nc.sync.dma_start(lt2[:], lt2_d[:])
            nc.sync.dma_start(ha[:], ha_d[:])
            nc.sync.dma_start(hb[:], hb_d[:])
            nc.sync.dma_start(lt3[:], lt3_d[:])

            def tslice(rg, half, sh):
                """(mov_tile, mov_col, stat_tile, stat_col, depth)"""
                Sd = 8 * half + sh
                if rg == 0:
                    n = int(pos0[Sd])
                    t = ha if n < NA else hb
                    mc = 256 * (n if n < NA else n - NA)
                    return t, mc, t, mc + 128, int(DEP[0, half, sh])
                t = lt1 if rg == 1 else (lt2 if rg <= 3 else lt3)
                off = {1: 0, 2: 0, 3: 4096, 4: 0, 5: 4096,
                       6: 8192, 7: 12288}[rg]
                return (t, off + 128 * Sd, t, off + 2048 + 128 * Sd,
                        int(DEP[rg, half, sh]))

            for rg in RGORDER:
                txm = []
                for half in range(2):
                    ps = psum.tile([128, 1024], F32, tag="ps",
                                   name=f"ps{rg}_{half}")
                    for sh in range(8):
                        t, mc, ts, sc, K = tslice(rg, half, sh)
                        nc.tensor.matmul(
                            ps[:, 128 * sh:128 * sh + 128],
                            ts[0:K, sc:sc + 128],
                            t[0:K, mc:mc + 128],
                            start=True, stop=True)
                    xs = xa[:, 1024 * half:1024 * (half + 1)]
                    tm = tpool.tile([128, 1024], FP16, tag="tx",
                                    name=f"tx{rg}_{half}")
                    if CFG_MODE[rg][half] == 'C':
                        cp = cpool.tile([128, 1024], FP16, tag="cp",
                                        name=f"cp{rg}_{half}")
                        nc.scalar.copy(cp[:], ps[:])
                        nc.vector.tensor_tensor(tm[:], cp[:], xs,
                                                mybir.AluOpType.mult)
                    else:
                        nc.vector.tensor_tensor(tm[:], ps[:], xs,
                                                mybir.AluOpType.mult)
                    txm.append(tm)
                t1 = rpool.tile([128, 1024], FP16, name=f"t1_{rg}",
                                tag="t1")
                nc.vector.tensor_tensor(t1[:, 0:VC], txm[0][:, 0:VC],
                                        txm[1][:, 0:VC],
                                        mybir.AluOpType.add)
                nc.gpsimd.tensor_tensor(t1[:, VC:1024], txm[0][:, VC:1024],
                                        txm[1][:, VC:1024],
                                        mybir.AluOpType.add)
                nc.sync.dma_start(out_d[128 * rg:128 * (rg + 1), :], t1[:])
    nc.compile()
    _cache["nc"] = nc
    return nc


def _pack_stat(P_):
    """Per-core tables with P folded into the stationary factors."""
    fac = _factors()
    G01, G2, Kmax01 = fac["G01"], fac["G2"], fac["Kmax01"]
    Pf = P_.reshape(M * M, O).astype(np.float32)
    statp = np.zeros((M * M, 32, 32), dtype=np.float32)
    statp[:128] = np.einsum('sg,sgkl->skl', Pf[:128], G01)
    statp[128:, 0:2] = np.einsum('sg,sgkl->skl', Pf[128:], G2)
    outs = []
    for c in range(NC):
        arrs = {k: v.copy() for k, v in fac["base"][c].items()}
        for name, (fi, si, ki, li) in fac["spec"][c].items():
            arrs[name].flat[fi] = statp[si, ki, li].astype(_bf16)
        outs.append(arrs)
    return outs


def _pack_x(x):
    # xa[32*ccol + l, 128*s + h] = x[32h+l, j], j = 4*s + ccol
    x4 = x.reshape(NH, NL, 16, 4)                 # [h, l, s, ccol]
    xa = np.ascontiguousarray(x4.transpose(3, 1, 2, 0)).reshape(128, 2048)
    return xa.astype(_fp16)


def _numpy_fallback(k, x, P_):
    out = np.zeros((B, M), dtype=np.float32)
    periods = (np.arange(M * M * O, dtype=np.float32) + 2.0).reshape(M, M, O)
    CH = 256
    for s0 in range(0, B, CH):
        kb = k[s0:s0 + CH].astype(np.float32)
        phi = np.cos(np.float32(TWO_PI) * kb[:, None, None, None]
                     / periods[None]).astype(np.float32)
        out[s0:s0 + CH] = np.einsum('bj,ijg,bijg->bi', x[s0:s0 + CH],
                                    P_.astype(np.float32), phi,
                                    optimize=True).astype(np.float32)
    return out


def kernel(k_tensor, token_indices, emb, P):
    global _last_results
    k = np.asarray(k_tensor, dtype=np.float32).reshape(B)
    tok = np.asarray(token_indices).astype(np.int64).reshape(B)
    emb_ = np.asarray(emb, dtype=np.float32)
    P_ = np.asarray(P, dtype=np.float32)
    x = emb_[tok]                                          # [B, 64]

    if not np.array_equal(k, np.arange(B, dtype=np.float32)):
        return _numpy_fallback(k, x, P_)

    fac = _factors()
    tabs = _pack_stat(P_)
    xa = _pack_x(x)
    nc = _build()
    in_maps = []
    for c in range(NC):
        m = {k2: np.ascontiguousarray(v) for k2, v in tabs[c].items()}
        m["xa"] = xa
        in_maps.append(m)
    res = run_bass_kernel_spmd(nc, in_maps, list(range(NC)))
    _last_results = res

    imap = fac["imap"]
    # out rows: 128*rg + 32*ccol + l; cols: 128*sh + h; b = 32h + l
    chains = []
    idx = []
    for c in range(NC):
        od = res.results[c]["out"].astype(np.float32)      # [1024, 1024]
        a = od.reshape(NI, 4, NL, 8, NH)                   # [rg,ccol,l,sh,h]
        a = a.transpose(0, 3, 1, 4, 2)                     # [rg,sh,ccol,h,l]
        chains.append(a.reshape(NI * 8 * 4, NH * NL))
        idx.append(imap[c].transpose(0, 1, 2).reshape(-1))  # [rg,sh,ccol]
    chains = np.concatenate(chains, axis=0)                # [2048, 4096]
    idx = np.concatenate(idx)
    order = np.argsort(idx, kind="stable")
    grouped = chains[order].reshape(M, 32, B).sum(axis=1)  # [i, b]
    return np.ascontiguousarray(grouped.T)                 # [b, i]
